# revision 3
# baseline (speedup 1.0000x reference)
"""GRU decoder kernel for Trainium2 (8 NeuronCores, data-parallel over batch).

Problem (hardcoded): B=4096, T=168, D=64, H=128.
  per step t:  gx_t = feats_t @ W_ih[:, :D].T + b_ih + y_prev * w_y
               gh   = h @ W_hh.T + b_hh
               r = sig(gx_r+gh_r); z = sig(gx_z+gh_z)
               n = tanh(gx_n + r*gh_n)
               h = (1-z)*n + z*h;  y = h @ wo + bo

Mapping per core: batch shard BS=512 split into TWO chains of C=256
columns, software-pipelined with a half-step offset so each chain's
serial step latency hides behind the other chain's engine work.

Per chain-step, layout [hidden dim on partitions, batch on free]:
  - feats arrive HOST-PRE-TRANSPOSED as [d, b] tiles (rows 0:64 = even t,
    64:128 = odd t).
  - rz psum bank [128, 512] (bufs=2) holds r|z: feats matmuls + a K=2
    ones-matmul adding the per-gate biases (K=4 with y0 feedback at t=0)
    + W1 @ h (W1 = W_hh + w_y (x) wo folds the y-feedback for t>=1).
    ONE fused ACT sigmoid (FD=512, no bias) -> r16|z16.
  - gh_n = whhnt @ h goes to its OWN shared psum bank (cols c2*256),
    so it depends only on h (runs concurrently with the sigmoid).
  - t1 = (gh_n + bhn) * r  (DVE STT); PE ident-matmul accumulates t1
    into the gxh psum (gx_n + wynt@h feedback); tanh(+bn bias) -> n.
  - zc=1-z, zh=z*h on gpsimd (off critical path); zn=zc*n, h'=zn+zh on DVE.
  - y_t = wo . h_t via M=32 matmul into a SHARED psum bank (chain A cols
    0:256, B 256:512), col-group t%4; every 4 steps one DVE copy
    evacuates [128,512] and one DMA writes yT[4g:4g+4, :] for both
    chains.  bo is added on the host after gather.

Emission order per step t (half-step chain offset):
  [even t: feats r/z + bias matmuls for (t, t+1), chunk DMA]
  C(A,t-1): zn,h' (DVE), y-mm
  A(A,t):   w1r,w1z,whhn,wyn (PE), fused sig (ACT)
  B(B,t-1): stt (DVE), ident (PE), tanh (ACT), zc/zh (gpsimd), feats_n(t)
  C(B,t-1) | A(B,t) | B(A,t)
"""

import numpy as np

import concourse.bacc as bacc
import concourse.bass as bass
import concourse.mybir as mybir
import concourse.tile as tile
from concourse.bass_utils import run_bass_kernel_spmd

B, T, D, H = 4096, 168, 64, 128
NCORES = 8
BS = B // NCORES  # 512
C = BS // 2       # 256 per chain
C2 = 2 * C

F32 = mybir.dt.float32
F16 = mybir.dt.float16
AF = mybir.ActivationFunctionType
ALU = mybir.AluOpType

CH = 12  # feats t-pairs per DMA chunk

# pack (fp16 [128, NPACK]) column layout
_WOC0 = 0       # [128, 32]   wo duplicated 32x
_ID0 = 32       # [128, 128]  identity (for t1 -> gx psum accumulate)
_FB = 160       # [128, 16] fp16 = [128, 8] fp32 bitcast: bhn, bn1, bn0
_RZONES = 176   # rows 0:2 AND 64:66, 512 cols: (1|0),(0|1) bias-mm rhs
_BRZ1 = 688     # rows 0:2 AND 64:66, 128 cols: br1, bz1 bias-mm lhsT
_K4L = 816      # rows 0:4, 128 cols: wy_r, wy_z, br0, bz0 (t=0 lhsT)
_K4R = 944      # rows 0:4, 512 cols per chain (x2): y0|0, 0|y0, 1|0, 0|1
_WYNR = 1968    # row 0, 128 cols: wy_n
_Y0R = 2096     # row 0, 256 cols per chain (x2): y0
NPACK = 2608


def build(nt=T):
    """Build the per-core Bass program. nt: number of timesteps (tests)."""
    assert nt % 4 == 0
    npairs = nt // 2
    ch = min(CH, npairs)
    nchunk = (npairs + ch - 1) // ch
    nc = bacc.Bacc("TRN2", target_bir_lowering=False, debug=False)

    featsT = nc.declare_dram_parameter("featsT", [256, npairs, C], F16, isOutput=False)
    h0T_d = nc.declare_dram_parameter("h0T", [128, BS], F16, isOutput=False)
    wft_d = nc.declare_dram_parameter("wft", [128, 384], F16, isOutput=False)
    w1t_d = nc.declare_dram_parameter("w1t", [128, 256], F16, isOutput=False)
    whhnt_d = nc.declare_dram_parameter("whhnt", [128, 128], F16, isOutput=False)
    wynt_d = nc.declare_dram_parameter("wynt", [128, 128], F16, isOutput=False)
    whhrz0_d = nc.declare_dram_parameter("whhrz0", [128, 256], F16, isOutput=False)
    pack = nc.declare_dram_parameter("pack", [128, NPACK], F16, isOutput=False)

    yT = nc.declare_dram_parameter("yT", [nt, BS], F16, isOutput=True)

    with tile.TileContext(nc) as tc:
        with (
            tc.tile_pool(name="wpool", bufs=1) as wpool,
            tc.tile_pool(name="fpool", bufs=2) as fpool,
            tc.tile_pool(name="hpool", bufs=2) as hpool,
            tc.tile_pool(name="gpool", bufs=2) as gpool,
            tc.tile_pool(name="ypool", bufs=2) as ypool,
            tc.tile_pool(name="ps_rz", bufs=2, space="PSUM") as ps_rz,
            tc.tile_pool(name="ps_gxh", bufs=1, space="PSUM") as ps_gxh,
            tc.tile_pool(name="ps_ghn", bufs=1, space="PSUM") as ps_ghn,
            tc.tile_pool(name="ps_u", bufs=1, space="PSUM") as ps_u,
        ):
            # ---- constants ----
            pk = wpool.tile([128, NPACK], F16)
            nc.sync.dma_start(pk[:], pack[:])
            woc = pk[:, _WOC0:_WOC0 + 32]
            ident = pk[:, _ID0:_ID0 + 128]
            fb = pk[:, _FB:_FB + 16].bitcast(F32)
            bhn, bn1, bn0 = fb[:, 0:1], fb[:, 1:2], fb[:, 2:3]
            rzones = [pk[0:2, _RZONES:_RZONES + 512],
                      pk[64:66, _RZONES:_RZONES + 512]]
            brz1 = [pk[0:2, _BRZ1:_BRZ1 + 128],
                    pk[64:66, _BRZ1:_BRZ1 + 128]]
            k4l = pk[0:4, _K4L:_K4L + 128]
            k4r = [pk[0:4, _K4R + 512 * c2:_K4R + 512 * (c2 + 1)] for c2 in (0, 1)]
            wynr = pk[0:1, _WYNR:_WYNR + 128]
            y0r = [pk[0:1, _Y0R + 256 * c2:_Y0R + 256 * (c2 + 1)] for c2 in (0, 1)]

            wft = wpool.tile([128, 384], F16)
            w1t = wpool.tile([128, 256], F16)
            whhnt = wpool.tile([128, 128], F16)
            wynt = wpool.tile([128, 128], F16)
            whhrz0 = wpool.tile([128, 256], F16)
            for sb, dr in [
                (wft, wft_d), (w1t, w1t_d), (whhnt, whhnt_d),
                (wynt, wynt_d), (whhrz0, whhrz0_d),
            ]:
                nc.sync.dma_start(sb[:], dr[:])
            h0sb = wpool.tile([128, BS], F16)
            nc.sync.dma_start(h0sb[:], h0T_d[:])

            hprev = [h0sb[:, 0:C], h0sb[:, C:2 * C]]

            # shared psum banks
            ghn = ps_ghn.tile([128, 512], F32)
            pus = ps_u.tile([128, 512], F32)

            # ---- feats chunks ----
            fchunks = [{}, {}]

            def load_chunk(c2, ci):
                p0 = ci * ch
                pn = min(ch, npairs - p0)
                ft = fpool.tile([128, ch * C], F16, tag=f"ft{c2}")
                nc.sync.dma_start(
                    ft[:, :pn * C], featsT[c2 * 128:(c2 + 1) * 128, p0:p0 + pn, :]
                )
                fchunks[c2][ci] = ft

            def fh_of(c2, tt):
                p = tt // 2
                ci, po = divmod(p, ch)
                half = (tt % 2) * 64
                return fchunks[c2][ci][half:half + 64, po * C:(po + 1) * C], half

            przs = [{}, {}]
            gxhs = [{}, {}]
            rz16s = [None, None]
            n16s = [None, None]
            zcs = [None, None]
            zhs = [None, None]

            def emit_feats_pair(c2, t):
                # rz-gate feats + bias matmuls for steps (t, t+1); even t
                # rows 0:64, odd t rows 64:128 run concurrently on
                # disjoint PE row groups.
                for tt in (t, t + 1):
                    if tt >= nt:
                        break
                    fh, half = fh_of(c2, tt)
                    w = wft[half:half + 64, :]
                    tp = (half, 0)
                    prz = ps_rz.tile([128, C2], F32, tag=f"rz{c2}")
                    nc.tensor.matmul(prz[:, 0:C], w[:, 0:128], fh,
                                     start=True, stop=False, tile_position=tp)
                    nc.tensor.matmul(prz[:, C:C2], w[:, 128:256], fh,
                                     start=False, stop=False, tile_position=tp)
                    if tt == 0:
                        # K=4: y0 feedback (wy_r, wy_z) + biases (br0, bz0)
                        nc.tensor.matmul(prz[:], k4l, k4r[c2],
                                         start=False, stop=False)
                    else:
                        par = tt % 2
                        nc.tensor.matmul(prz[:], brz1[par], rzones[par],
                                         start=False, stop=False,
                                         tile_position=(par * 64, 0))
                    przs[c2][tt] = prz

            def emit_feats_n(c2, tt):
                # n-gate feats for step tt into the gxh bank (bufs=1: the
                # start=True write waits for tanh(tt-1)'s read -- emitted
                # right after it).
                fh, half = fh_of(c2, tt)
                gxh = ps_gxh.tile([128, C], F32, tag=f"gxh{c2}")
                nc.tensor.matmul(gxh[:], wft[half:half + 64, 256:384], fh,
                                 start=True, stop=False, tile_position=(half, 0))
                if tt == 0:
                    nc.tensor.matmul(gxh[:], wynr, y0r[c2],
                                     start=False, stop=False)
                gxhs[c2][tt] = gxh

            def stage_a(c2, t):
                # recurrent matmul burst + fused sigmoid
                hp = hprev[c2]
                prz = przs[c2].pop(t)
                wrz = whhrz0 if t == 0 else w1t
                nc.tensor.matmul(prz[:, 0:C], wrz[:, 0:128], hp,
                                 start=False, stop=False)
                nc.tensor.matmul(prz[:, C:C2], wrz[:, 128:256], hp,
                                 start=False, stop=True)
                nc.tensor.matmul(ghn[:, c2 * C:(c2 + 1) * C], whhnt[:], hp,
                                 start=True, stop=True)
                if t > 0:
                    nc.tensor.matmul(gxhs[c2][t], wynt[:], hp,
                                     start=False, stop=False)
                rz = gpool.tile([128, C2], F16, tag=f"rz16{c2}")
                nc.scalar.activation(rz[:], prz[:], AF.Sigmoid)
                rz16s[c2] = rz

            def stage_b(c2, t):
                # t1, ident-accumulate, tanh, zc/zh; prefetch feats_n(t+1)
                rz = rz16s[c2]
                t1 = gpool.tile([128, C], F16, tag=f"t1{c2}")
                nc.vector.scalar_tensor_tensor(
                    t1[:], ghn[:, c2 * C:(c2 + 1) * C], bhn, rz[:, 0:C],
                    ALU.add, ALU.mult)
                gxh = gxhs[c2].pop(t)
                nc.tensor.matmul(gxh[:], ident, t1[:],
                                 start=False, stop=True)
                n16 = gpool.tile([128, C], F16, tag=f"n16{c2}")
                nc.scalar.activation(n16[:], gxh[:], AF.Tanh,
                                     bias=bn0 if t == 0 else bn1)
                n16s[c2] = n16
                zc = gpool.tile([128, C], F16, tag=f"zc{c2}")
                nc.gpsimd.tensor_scalar(zc[:], rz[:, C:C2], -1.0, 1.0,
                                        ALU.mult, ALU.add)
                zh = gpool.tile([128, C], F16, tag=f"zh{c2}")
                nc.gpsimd.tensor_tensor(zh[:], rz[:, C:C2], hprev[c2], ALU.mult)
                zcs[c2] = zc
                zhs[c2] = zh
                if t + 1 < nt:
                    emit_feats_n(c2, t + 1)

            def stage_c(c2, t):
                # combine h' and the y matmul; evac every 4 steps
                zn = gpool.tile([128, C], F16, tag=f"zn{c2}")
                nc.vector.tensor_tensor(zn[:], zcs[c2][:], n16s[c2][:], ALU.mult)
                hT = hpool.tile([128, C], F16, tag=f"h{c2}")
                nc.vector.tensor_tensor(hT[:], zn[:], zhs[c2][:], ALU.add)
                hprev[c2] = hT
                c4 = t % 4
                nc.tensor.matmul(
                    pus[32 * c4:32 * (c4 + 1), c2 * C:(c2 + 1) * C], woc, hT[:],
                    start=True, stop=True, tile_position=(0, 32 * c4),
                )
                if c4 == 3 and c2 == 1:
                    g = t // 4
                    yf = ypool.tile([128, 512], F16, tag="yf")
                    nc.vector.tensor_copy(yf[:], pus[:])
                    nc.sync.dma_start(yT[4 * g:4 * (g + 1), :], yf[0:128:32, :])

            # ---- prologue ----
            for c2 in (0, 1):
                load_chunk(c2, 0)
                if nchunk > 1:
                    load_chunk(c2, 1)
                emit_feats_pair(c2, 0)
                emit_feats_n(c2, 0)

            # ---- steady loop ----
            for t in range(nt):
                if t % 2 == 0 and t > 0:
                    p = t // 2
                    ci = p // ch
                    for c2 in (0, 1):
                        if p % ch == 0 and ci + 1 < nchunk:
                            load_chunk(c2, ci + 1)
                        emit_feats_pair(c2, t)
                if t > 0:
                    stage_c(0, t - 1)
                stage_a(0, t)
                if t > 0:
                    stage_b(1, t - 1)
                    stage_c(1, t - 1)
                stage_a(1, t)
                stage_b(0, t)

            # ---- tail ----
            stage_b(1, nt - 1)
            stage_c(0, nt - 1)
            stage_c(1, nt - 1)

    nc.compile()
    return nc


# -------- host-side weight prep + sharded execution --------

def _prep_aux(W_ih, W_hh, b_ih, b_hh, Wo, bo):
    W_ih = np.asarray(W_ih, np.float32)
    W_hh = np.asarray(W_hh, np.float32)
    b_ih = np.asarray(b_ih, np.float32)
    b_hh = np.asarray(b_hh, np.float32)
    wo = np.asarray(Wo, np.float32)[0]       # [H]
    bo_s = float(np.asarray(bo, np.float32)[0])
    wfd = W_ih[:, :D]                         # [3H, D]
    w_y = W_ih[:, D]                          # [3H]

    wft = np.zeros((128, 384), np.float16)
    wft[0:64] = wfd.T.astype(np.float16)
    wft[64:128] = wfd.T.astype(np.float16)

    W1 = W_hh[0:2 * H] + np.outer(w_y[0:2 * H], wo)       # [2H, H]
    aux = dict(
        wft=wft,
        w1t=np.ascontiguousarray(W1.T.astype(np.float16)),
        whhnt=np.ascontiguousarray(W_hh[2 * H:].T.astype(np.float16)),
        wynt=np.ascontiguousarray(np.outer(wo, w_y[2 * H:]).astype(np.float16)),
        whhrz0=np.ascontiguousarray(W_hh[0:2 * H].T.astype(np.float16)),
    )

    pk = np.zeros((128, NPACK), np.float16)
    pk[:, _WOC0:_WOC0 + 32] = np.repeat(wo[:, None], 32, axis=1).astype(np.float16)
    pk[:, _ID0:_ID0 + 128] = np.eye(128, dtype=np.float16)
    fbv = np.stack(
        [b_hh[2 * H:],                          # bhn
         b_ih[2 * H:] + w_y[2 * H:] * bo_s,     # bn1
         b_ih[2 * H:],                          # bn0
         np.zeros(128, np.float32),
         np.zeros(128, np.float32),
         np.zeros(128, np.float32),
         np.zeros(128, np.float32),
         np.zeros(128, np.float32)],
        axis=1,
    ).astype(np.float32)
    pk[:, _FB:_FB + 16] = np.ascontiguousarray(fbv).view(np.float16)

    brz_base = (b_ih + b_hh)[0:2 * H]
    br1 = (brz_base[0:H] + w_y[0:H] * bo_s).astype(np.float16)
    bz1 = (brz_base[H:2 * H] + w_y[H:2 * H] * bo_s).astype(np.float16)
    for p0 in (0, 64):
        pk[p0 + 0, _RZONES:_RZONES + 256] = 1.0
        pk[p0 + 1, _RZONES + 256:_RZONES + 512] = 1.0
        pk[p0 + 0, _BRZ1:_BRZ1 + 128] = br1
        pk[p0 + 1, _BRZ1:_BRZ1 + 128] = bz1
    pk[0, _K4L:_K4L + 128] = w_y[0:H].astype(np.float16)
    pk[1, _K4L:_K4L + 128] = w_y[H:2 * H].astype(np.float16)
    pk[2, _K4L:_K4L + 128] = brz_base[0:H].astype(np.float16)
    pk[3, _K4L:_K4L + 128] = brz_base[H:2 * H].astype(np.float16)
    # K4R ones rows (y0 rows filled per-core in _fill_y0)
    for c2 in (0, 1):
        base = _K4R + 512 * c2
        pk[2, base:base + 256] = 1.0
        pk[3, base + 256:base + 512] = 1.0
    pk[0, _WYNR:_WYNR + 128] = w_y[2 * H:].astype(np.float16)
    aux["pack"] = pk
    aux["bo_s"] = bo_s
    return aux


def _core_featsT(ff_core):
    """[BS, nt, D] fp16 -> [256, nt//2, C]: rows = c2*128 + (t%2)*64 + d."""
    nt = ff_core.shape[1]
    a = ff_core.reshape(2, C, nt // 2, 2, D).transpose(0, 3, 4, 2, 1)
    return np.ascontiguousarray(a).reshape(256, nt // 2, C)


def _fill_y0(pkc, y0c):
    """Write per-core y0 (fp16 [BS]) into the pack's K4R / y0r slots."""
    for c2 in (0, 1):
        sl = y0c[c2 * C:(c2 + 1) * C]
        base = _K4R + 512 * c2
        pkc[0, base:base + 256] = sl
        pkc[1, base + 256:base + 512] = sl
        pkc[0, _Y0R + 256 * c2:_Y0R + 256 * (c2 + 1)] = sl


_NC_CACHE = {}


def kernel(future_feats, h0, y0, W_ih, W_hh, b_ih, b_hh, Wo, bo):
    ff = np.asarray(future_feats).astype(np.float16)      # [B, T, D]
    h0f = np.asarray(h0).astype(np.float16)[0]            # [B, H]
    y0f = np.asarray(y0).astype(np.float16)               # [B]

    aux = _prep_aux(W_ih, W_hh, b_ih, b_hh, Wo, bo)
    bo_s = aux.pop("bo_s")

    if "nc" not in _NC_CACHE:
        _NC_CACHE["nc"] = build(T)
    nc = _NC_CACHE["nc"]

    in_maps = []
    for c in range(NCORES):
        sl = slice(c * BS, (c + 1) * BS)
        m = dict(aux)
        pkc = aux["pack"].copy()
        _fill_y0(pkc, y0f[sl])
        m["pack"] = pkc
        m["featsT"] = _core_featsT(ff[sl])
        m["h0T"] = np.ascontiguousarray(h0f[sl].T)
        in_maps.append(m)

    res = run_bass_kernel_spmd(nc, in_maps, core_ids=list(range(NCORES)))
    outs = [r["yT"] for r in res.results]
    out = np.concatenate([o.T.astype(np.float32) for o in outs], axis=0)
    return out + bo_s


# revision 12
# speedup vs baseline: 1.3001x; 1.3001x over previous
"""GRU decoder kernel for Trainium2 (8 NeuronCores, data-parallel over batch).

Problem (hardcoded): B=4096, T=168, D=64, H=128.
  per step t:  gx_t = feats_t @ W_ih[:, :D].T + b_ih + y_prev * w_y
               gh   = h @ W_hh.T + b_hh
               r = sig(gx_r+gh_r); z = sig(gx_z+gh_z)
               n = tanh(gx_n + r*gh_n)
               h = (1-z)*n + z*h;  y = h @ wo + bo

Mapping per core: batch shard BS=512 split into TWO chains of C=256
columns, software-pipelined with a half-step offset so one chain's
serial step latency hides behind the other's engine work.

Layout [hidden dim on partitions, batch on free].  PSUM banks are per
GATE, merged across chains (cols 0:256 chain A, 256:512 chain B):
  pr, pz, pn  [128,512] bufs=2   r / z / n pre-activations
  ghn         [128,512] shared   gh_n = whhnt @ h per chain half
  pus         [128,512] shared   y accumulation (32 rows per t%4)
This lets ONE feats matmul (N=512, K=64 row-packed by t parity) feed
both chains, and drops all bias matmuls: each per-gate sigmoid/tanh is
a per-chain ACT op (FD=256) with a per-partition bias vector.
W1 = W_hh + w_y (x) wo folds the y-feedback for t>=1 (K=1 matmuls
against the supplied y0 cover t=0).

Per chain-step: burst [w1r, w1z, whhn, wyn] -> sig_r, sig_z (ACT) ->
t1 = (gh_n+bhn)*r (DVE STT) -> PE ident-matmul accumulates t1 into pn
-> tanh (ACT) -> zc=1-z, zh=z*h (gpsimd, off path) -> zn=zc*n,
h'=zn+zh (DVE) -> y matmul.  Every 4 steps one DVE copy evacuates pus
and one DMA writes yT[4g:4g+4, :].  bo is added on the host.

A ~5us dense dummy-matmul burst at kernel start forces the PE HAM
clock gate to 8/8 (2.4 GHz) before the loop begins.
"""

import numpy as np

import concourse.bacc as bacc
import concourse.bass as bass
import concourse.mybir as mybir
import concourse.tile as tile
from concourse.bass_utils import run_bass_kernel_spmd

B, T, D, H = 4096, 168, 64, 128
NCORES = 8
BS = B // NCORES  # 512
C = BS // 2       # 256 per chain

F32 = mybir.dt.float32
F16 = mybir.dt.float16
AF = mybir.ActivationFunctionType
ALU = mybir.AluOpType

CH = 12      # feats t-pairs per DMA chunk
NWARM = 24   # dummy matmuls to warm the PE HAM clock gate

# pack (fp16 [128, NPACK]) column layout
_WOC0 = 0       # [128, 32]   wo duplicated 32x
_ID0 = 32       # [128, 128]  identity (for t1 -> pn psum accumulate)
_FB = 160       # [128, 16] fp16 = [128, 8] fp32 bitcast:
                #   bhn, bn1, bn0, br1, bz1, br0, bz0
_WYR = 176      # row 0, 128 cols: wy_r   (t=0 y0-feedback lhsT)
_WYZ = 304      # row 0, 128 cols: wy_z
_WYN = 432      # row 0, 128 cols: wy_n
_Y0AB = 560     # row 0, 512 cols: y0 (chain A | chain B)
NPACK = 1072


def build(nt=T):
    """Build the per-core Bass program. nt: number of timesteps (tests)."""
    assert nt % 4 == 0
    npairs = nt // 2
    ch = min(CH, npairs)
    nchunk = (npairs + ch - 1) // ch
    nc = bacc.Bacc("TRN2", target_bir_lowering=False, debug=False)

    featsT = nc.declare_dram_parameter("featsT", [128, npairs, 512], F16, isOutput=False)
    h0T_d = nc.declare_dram_parameter("h0T", [128, BS], F16, isOutput=False)
    wft_d = nc.declare_dram_parameter("wft", [128, 384], F16, isOutput=False)
    w1t_d = nc.declare_dram_parameter("w1t", [128, 256], F16, isOutput=False)
    whhnt_d = nc.declare_dram_parameter("whhnt", [128, 128], F16, isOutput=False)
    wynt_d = nc.declare_dram_parameter("wynt", [128, 128], F16, isOutput=False)
    whhrz0_d = nc.declare_dram_parameter("whhrz0", [128, 256], F16, isOutput=False)
    pack = nc.declare_dram_parameter("pack", [128, NPACK], F16, isOutput=False)

    yT = nc.declare_dram_parameter("yT", [nt, BS], F16, isOutput=True)

    with tile.TileContext(nc) as tc:
        with (
            tc.tile_pool(name="wpool", bufs=1) as wpool,
            tc.tile_pool(name="fpool", bufs=2) as fpool,
            tc.tile_pool(name="hpool", bufs=2) as hpool,
            tc.tile_pool(name="gpool", bufs=2) as gpool,
            tc.tile_pool(name="ypool", bufs=2) as ypool,
            tc.tile_pool(name="ps_r", bufs=2, space="PSUM") as ps_r,
            tc.tile_pool(name="ps_z", bufs=2, space="PSUM") as ps_z,
            tc.tile_pool(name="ps_n", bufs=2, space="PSUM") as ps_n,
            tc.tile_pool(name="ps_ghn", bufs=1, space="PSUM") as ps_ghn,
            tc.tile_pool(name="ps_u", bufs=1, space="PSUM") as ps_u,
        ):
            # ---- constants ----
            pk = wpool.tile([128, NPACK], F16)
            nc.sync.dma_start(pk[:], pack[:])
            woc = pk[:, _WOC0:_WOC0 + 32]
            ident = pk[:, _ID0:_ID0 + 128]
            fb = pk[:, _FB:_FB + 16].bitcast(F32)
            bhn, bn1, bn0 = fb[:, 0:1], fb[:, 1:2], fb[:, 2:3]
            br1, bz1 = fb[:, 3:4], fb[:, 4:5]
            br0, bz0 = fb[:, 5:6], fb[:, 6:7]
            wyr = pk[0:1, _WYR:_WYR + 128]
            wyz = pk[0:1, _WYZ:_WYZ + 128]
            wyn = pk[0:1, _WYN:_WYN + 128]
            y0ab = pk[0:1, _Y0AB:_Y0AB + 512]

            wft = wpool.tile([128, 384], F16)
            w1t = wpool.tile([128, 256], F16)
            whhnt = wpool.tile([128, 128], F16)
            wynt = wpool.tile([128, 128], F16)
            whhrz0 = wpool.tile([128, 256], F16)
            for sb, dr in [
                (wft, wft_d), (w1t, w1t_d), (whhnt, whhnt_d),
                (wynt, wynt_d), (whhrz0, whhrz0_d),
            ]:
                nc.sync.dma_start(sb[:], dr[:])
            h0sb = wpool.tile([128, BS], F16)
            nc.sync.dma_start(h0sb[:], h0T_d[:])

            hprev = [h0sb[:, 0:C], h0sb[:, C:2 * C]]

            # shared psum banks
            ghn = ps_ghn.tile([128, 512], F32)
            pus = ps_u.tile([128, 512], F32)

            # ---- HAM warm-up: dense dummy matmuls into pus ----
            for _ in range(NWARM):
                nc.tensor.matmul(pus[:], ident, pk[:, 0:512],
                                 start=True, stop=True)

            # ---- feats chunks ----
            fchunks = {}

            def load_chunk(ci):
                p0 = ci * ch
                pn_ = min(ch, npairs - p0)
                ft = fpool.tile([128, ch * 512], F16, tag="ft")
                nc.sync.dma_start(ft[:, :pn_ * 512], featsT[:, p0:p0 + pn_, :])
                fchunks[ci] = ft

            def fh_of(tt):
                p = tt // 2
                ci, po = divmod(p, ch)
                half = (tt % 2) * 64
                return fchunks[ci][half:half + 64, po * 512:(po + 1) * 512], half

            prs, pzs, pns = {}, {}, {}
            r16s = [None, None]
            z16s = [None, None]
            n16s = [None, None]
            zcs = [None, None]
            zhs = [None, None]

            def emit_feats_pair(t):
                # per-gate feats matmuls for steps (t, t+1), both chains in
                # one N=512 matmul; even t rows 0:64, odd rows 64:128.
                for tt in (t, t + 1):
                    if tt >= nt:
                        break
                    fh, half = fh_of(tt)
                    w = wft[half:half + 64, :]
                    tp = (half, 0)
                    pr = ps_r.tile([128, 512], F32, tag="pr")
                    pz = ps_z.tile([128, 512], F32, tag="pz")
                    pn = ps_n.tile([128, 512], F32, tag="pn")
                    # merged-chain N=512 matmuls; the sim's zero-region
                    # group checker can't express "open bank, per-chain
                    # sub-region accumulates, reads of finished halves",
                    # but per-element has_written bits make it correct on
                    # HW -> skip_group_check on every pr/pz/pn matmul.
                    nc.tensor.matmul(pr[:], w[:, 0:128], fh, start=True,
                                     stop=False, tile_position=tp,
                                     skip_group_check=True)
                    nc.tensor.matmul(pz[:], w[:, 128:256], fh, start=True,
                                     stop=False, tile_position=tp,
                                     skip_group_check=True)
                    nc.tensor.matmul(pn[:], w[:, 256:384], fh, start=True,
                                     stop=False, tile_position=tp,
                                     skip_group_check=True)
                    if tt == 0:
                        nc.tensor.matmul(pr[:], wyr, y0ab, start=False,
                                         stop=False, skip_group_check=True)
                        nc.tensor.matmul(pz[:], wyz, y0ab, start=False,
                                         stop=False, skip_group_check=True)
                        nc.tensor.matmul(pn[:], wyn, y0ab, start=False,
                                         stop=False, skip_group_check=True)
                    prs[tt] = pr
                    pzs[tt] = pz
                    pns[tt] = pn

            def stage_a(c2, t):
                # recurrent matmul burst + per-gate sigmoids
                hp = hprev[c2]
                cs = slice(c2 * C, (c2 + 1) * C)
                wrz = whhrz0 if t == 0 else w1t
                nc.tensor.matmul(prs[t][:, cs], wrz[:, 0:128], hp,
                                 start=False, stop=(c2 == 1),
                                 skip_group_check=True)
                nc.tensor.matmul(pzs[t][:, cs], wrz[:, 128:256], hp,
                                 start=False, stop=(c2 == 1),
                                 skip_group_check=True)
                nc.tensor.matmul(ghn[:, cs], whhnt[:], hp,
                                 start=True, stop=True)
                if t > 0:
                    nc.tensor.matmul(pns[t][:, cs], wynt[:], hp,
                                     start=False, stop=False,
                                     skip_group_check=True)
                r16 = gpool.tile([128, C], F16, tag=f"r16{c2}")
                nc.scalar.activation(r16[:], prs[t][:, cs], AF.Sigmoid,
                                     bias=br0 if t == 0 else br1)
                z16 = gpool.tile([128, C], F16, tag=f"z16{c2}")
                nc.scalar.activation(z16[:], pzs[t][:, cs], AF.Sigmoid,
                                     bias=bz0 if t == 0 else bz1)
                r16s[c2] = r16
                z16s[c2] = z16
                if c2 == 1:
                    prs.pop(t)
                    pzs.pop(t)

            def stage_b(c2, t):
                # t1, ident-accumulate, tanh, zc/zh
                cs = slice(c2 * C, (c2 + 1) * C)
                t1 = gpool.tile([128, C], F16, tag=f"t1{c2}")
                nc.vector.scalar_tensor_tensor(
                    t1[:], ghn[:, cs], bhn, r16s[c2][:], ALU.add, ALU.mult)
                pn = pns[t]
                nc.tensor.matmul(pn[:, cs], ident, t1[:],
                                 start=False, stop=(c2 == 1),
                                 skip_group_check=True)
                n16 = gpool.tile([128, C], F16, tag=f"n16{c2}")
                nc.scalar.activation(n16[:], pn[:, cs], AF.Tanh,
                                     bias=bn0 if t == 0 else bn1)
                n16s[c2] = n16
                zc = gpool.tile([128, C], F16, tag=f"zc{c2}")
                nc.gpsimd.tensor_scalar(zc[:], z16s[c2][:], -1.0, 1.0,
                                        ALU.mult, ALU.add)
                zh = gpool.tile([128, C], F16, tag=f"zh{c2}")
                nc.gpsimd.tensor_tensor(zh[:], z16s[c2][:], hprev[c2], ALU.mult)
                zcs[c2] = zc
                zhs[c2] = zh
                if c2 == 1:
                    pns.pop(t)

            def stage_c(c2, t):
                # combine h' and the y matmul; evac every 4 steps
                zn = gpool.tile([128, C], F16, tag=f"zn{c2}")
                nc.vector.tensor_tensor(zn[:], zcs[c2][:], n16s[c2][:], ALU.mult)
                hT = hpool.tile([128, C], F16, tag=f"h{c2}")
                nc.vector.tensor_tensor(hT[:], zn[:], zhs[c2][:], ALU.add)
                hprev[c2] = hT
                c4 = t % 4
                nc.tensor.matmul(
                    pus[32 * c4:32 * (c4 + 1), c2 * C:(c2 + 1) * C], woc, hT[:],
                    start=True, stop=True, tile_position=(0, 32 * c4),
                )
                if c4 == 3 and c2 == 1:
                    g = t // 4
                    yf = ypool.tile([128, 512], F16, tag="yf")
                    nc.vector.tensor_copy(yf[:], pus[:])
                    nc.sync.dma_start(yT[4 * g:4 * (g + 1), :], yf[0:128:32, :])

            # ---- prologue ----
            load_chunk(0)
            if nchunk > 1:
                load_chunk(1)
            emit_feats_pair(0)

            # ---- steady loop ----
            for t in range(nt):
                if t % 2 == 0 and t > 0:
                    p = t // 2
                    ci = p // ch
                    if p % ch == 0 and ci + 1 < nchunk:
                        load_chunk(ci + 1)
                    emit_feats_pair(t)
                if t > 0:
                    stage_b(1, t - 1)
                    stage_c(0, t - 1)
                stage_a(0, t)
                if t > 0:
                    stage_c(1, t - 1)
                stage_a(1, t)
                stage_b(0, t)

            # ---- tail ----
            stage_b(1, nt - 1)
            stage_c(0, nt - 1)
            stage_c(1, nt - 1)

    nc.compile()
    return nc


# -------- host-side weight prep + sharded execution --------

def _prep_aux(W_ih, W_hh, b_ih, b_hh, Wo, bo):
    W_ih = np.asarray(W_ih, np.float32)
    W_hh = np.asarray(W_hh, np.float32)
    b_ih = np.asarray(b_ih, np.float32)
    b_hh = np.asarray(b_hh, np.float32)
    wo = np.asarray(Wo, np.float32)[0]       # [H]
    bo_s = float(np.asarray(bo, np.float32)[0])
    wfd = W_ih[:, :D]                         # [3H, D]
    w_y = W_ih[:, D]                          # [3H]

    wft = np.zeros((128, 384), np.float16)
    wft[0:64] = wfd.T.astype(np.float16)
    wft[64:128] = wfd.T.astype(np.float16)

    W1 = W_hh[0:2 * H] + np.outer(w_y[0:2 * H], wo)       # [2H, H]
    aux = dict(
        wft=wft,
        w1t=np.ascontiguousarray(W1.T.astype(np.float16)),
        whhnt=np.ascontiguousarray(W_hh[2 * H:].T.astype(np.float16)),
        wynt=np.ascontiguousarray(np.outer(wo, w_y[2 * H:]).astype(np.float16)),
        whhrz0=np.ascontiguousarray(W_hh[0:2 * H].T.astype(np.float16)),
    )

    pk = np.zeros((128, NPACK), np.float16)
    pk[:, _WOC0:_WOC0 + 32] = np.repeat(wo[:, None], 32, axis=1).astype(np.float16)
    pk[:, _ID0:_ID0 + 128] = np.eye(128, dtype=np.float16)
    brz_base = (b_ih + b_hh)[0:2 * H]
    fbv = np.stack(
        [b_hh[2 * H:],                            # bhn
         b_ih[2 * H:] + w_y[2 * H:] * bo_s,       # bn1
         b_ih[2 * H:],                            # bn0
         brz_base[0:H] + w_y[0:H] * bo_s,         # br1
         brz_base[H:2 * H] + w_y[H:2 * H] * bo_s,  # bz1
         brz_base[0:H],                           # br0
         brz_base[H:2 * H]],                      # bz0
        axis=1,
    ).astype(np.float32)
    fb16 = np.zeros((128, 16), np.float16)
    fb16[:, 0:14] = fbv.view(np.float16)
    pk[:, _FB:_FB + 16] = fb16
    pk[0, _WYR:_WYR + 128] = w_y[0:H].astype(np.float16)
    pk[0, _WYZ:_WYZ + 128] = w_y[H:2 * H].astype(np.float16)
    pk[0, _WYN:_WYN + 128] = w_y[2 * H:].astype(np.float16)
    aux["pack"] = pk
    aux["bo_s"] = bo_s
    return aux


def _core_featsT(ff_core):
    """[BS, nt, D] fp16 -> [128, nt//2, 512]:
    rows = (t%2)*64 + d, cols = c2*256 + batch-within-chain."""
    nt = ff_core.shape[1]
    a = ff_core.reshape(2, C, nt // 2, 2, D)   # [c2, cb, p, par, d]
    a = a.transpose(3, 4, 2, 0, 1)             # [par, d, p, c2, cb]
    return np.ascontiguousarray(a).reshape(128, nt // 2, 512)


def _fill_y0(pkc, y0c):
    """Write per-core y0 (fp16 [BS]) into the pack's y0 row."""
    pkc[0, _Y0AB:_Y0AB + 512] = y0c


_NC_CACHE = {}


def kernel(future_feats, h0, y0, W_ih, W_hh, b_ih, b_hh, Wo, bo):
    ff = np.asarray(future_feats).astype(np.float16)      # [B, T, D]
    h0f = np.asarray(h0).astype(np.float16)[0]            # [B, H]
    y0f = np.asarray(y0).astype(np.float16)               # [B]

    aux = _prep_aux(W_ih, W_hh, b_ih, b_hh, Wo, bo)
    bo_s = aux.pop("bo_s")

    if "nc" not in _NC_CACHE:
        _NC_CACHE["nc"] = build(T)
    nc = _NC_CACHE["nc"]

    in_maps = []
    for c in range(NCORES):
        sl = slice(c * BS, (c + 1) * BS)
        m = dict(aux)
        pkc = aux["pack"].copy()
        _fill_y0(pkc, y0f[sl])
        m["pack"] = pkc
        m["featsT"] = _core_featsT(ff[sl])
        m["h0T"] = np.ascontiguousarray(h0f[sl].T)
        in_maps.append(m)

    res = run_bass_kernel_spmd(nc, in_maps, core_ids=list(range(NCORES)))
    outs = [r["yT"] for r in res.results]
    out = np.concatenate([o.T.astype(np.float32) for o in outs], axis=0)
    return out + bo_s


# revision 21
# speedup vs baseline: 1.3001x; 1.0000x over previous
"""GRU decoder kernel for Trainium2 (8 NeuronCores, data-parallel over batch).

Problem (hardcoded): B=4096, T=168, D=64, H=128.
  per step t:  gx_t = feats_t @ W_ih[:, :D].T + b_ih + y_prev * w_y
               gh   = h @ W_hh.T + b_hh
               r = sig(gx_r+gh_r); z = sig(gx_z+gh_z)
               n = tanh(gx_n + r*gh_n)
               h = (1-z)*n + z*h;  y = h @ wo + bo

Mapping per core: batch shard BS=512 split into TWO chains of C=256
columns, software-pipelined with a half-step offset so one chain's
serial step latency hides behind the other's engine work.

Layout [hidden dim on partitions, batch on free].  PSUM banks are per
GATE, merged across chains (cols 0:256 chain A, 256:512 chain B):
  pr, pz, pn  [128,512] bufs=2   r / z / n pre-activations
  ghn         [128,512] shared   gh_n = whhnt @ h per chain half
  pus         [128,512] shared   y accumulation (32 rows per t%4)
This lets ONE feats matmul (N=512, K=64 row-packed by t parity) feed
both chains, and drops all bias matmuls: each per-gate sigmoid/tanh is
a per-chain ACT op (FD=256) with a per-partition bias vector.
W1 = W_hh + w_y (x) wo folds the y-feedback for t>=1 (K=1 matmuls
against the supplied y0 cover t=0).

Per chain-step: burst [w1r, w1z, whhn, wyn] -> sig_r, sig_z (ACT) ->
t1 = (gh_n+bhn)*r (DVE STT) -> PE ident-matmul accumulates t1 into pn
-> tanh (ACT) -> zc=1-z, zh=z*h (gpsimd, off path) -> zn=zc*n,
h'=zn+zh (DVE) -> y matmul.  Every 4 steps one DVE copy evacuates pus
and one DMA writes yT[4g:4g+4, :].  bo is added on the host.

A ~5us dense dummy-matmul burst at kernel start forces the PE HAM
clock gate to 8/8 (2.4 GHz) before the loop begins.
"""

import numpy as np

import concourse.bacc as bacc
import concourse.bass as bass
import concourse.mybir as mybir
import concourse.tile as tile
from concourse.bass_utils import run_bass_kernel_spmd

B, T, D, H = 4096, 168, 64, 128
NCORES = 8
BS = B // NCORES  # 512
C = BS // 2       # 256 per chain

F32 = mybir.dt.float32
F16 = mybir.dt.float16
AF = mybir.ActivationFunctionType
ALU = mybir.AluOpType

CH = 12      # feats t-pairs per DMA chunk
NWARM = 24   # dummy matmuls to warm the PE HAM clock gate
NFILL_A = 0  # zero-matmul fillers before each recurrent burst
NFILL_B = 0  # zero-matmul fillers before each ident accumulate

# pack (fp16 [128, NPACK]) column layout
_WOC0 = 0       # [128, 32]   wo duplicated 32x
_ID0 = 32       # [128, 128]  identity (for t1 -> pn psum accumulate)
_FB = 160       # [128, 16] fp16 = [128, 8] fp32 bitcast:
                #   bhn, bn1, bn0, br1, bz1, br0, bz0
_WYR = 176      # row 0, 128 cols: wy_r   (t=0 y0-feedback lhsT)
_WYZ = 304      # row 0, 128 cols: wy_z
_WYN = 432      # row 0, 128 cols: wy_n
_Y0AB = 560     # row 0, 512 cols: y0 (chain A | chain B)
NPACK = 1072


def build(nt=T):
    """Build the per-core Bass program. nt: number of timesteps (tests)."""
    assert nt % 4 == 0
    npairs = nt // 2
    ch = min(CH, npairs)
    nchunk = (npairs + ch - 1) // ch
    nc = bacc.Bacc("TRN2", target_bir_lowering=False, debug=False)

    featsT = nc.declare_dram_parameter("featsT", [128, npairs, 512], F16, isOutput=False)
    h0T_d = nc.declare_dram_parameter("h0T", [128, BS], F16, isOutput=False)
    wft_d = nc.declare_dram_parameter("wft", [128, 384], F16, isOutput=False)
    w1t_d = nc.declare_dram_parameter("w1t", [128, 256], F16, isOutput=False)
    whhnt_d = nc.declare_dram_parameter("whhnt", [128, 128], F16, isOutput=False)
    wynt_d = nc.declare_dram_parameter("wynt", [128, 128], F16, isOutput=False)
    whhrz0_d = nc.declare_dram_parameter("whhrz0", [128, 256], F16, isOutput=False)
    pack = nc.declare_dram_parameter("pack", [128, NPACK], F16, isOutput=False)

    yT = nc.declare_dram_parameter("yT", [nt, BS], F16, isOutput=True)

    with tile.TileContext(nc) as tc:
        with (
            tc.tile_pool(name="wpool", bufs=1) as wpool,
            tc.tile_pool(name="fpool", bufs=2) as fpool,
            tc.tile_pool(name="hpool", bufs=2) as hpool,
            tc.tile_pool(name="gpool", bufs=2) as gpool,
            tc.tile_pool(name="ypool", bufs=2) as ypool,
            tc.tile_pool(name="ps_r", bufs=2, space="PSUM") as ps_r,
            tc.tile_pool(name="ps_z", bufs=2, space="PSUM") as ps_z,
            tc.tile_pool(name="ps_n", bufs=2, space="PSUM") as ps_n,
            tc.tile_pool(name="ps_ghn", bufs=1, space="PSUM") as ps_ghn,
            tc.tile_pool(name="ps_u", bufs=1, space="PSUM") as ps_u,
        ):
            # ---- constants ----
            pk = wpool.tile([128, NPACK], F16)
            nc.sync.dma_start(pk[:], pack[:])
            woc = pk[:, _WOC0:_WOC0 + 32]
            ident = pk[:, _ID0:_ID0 + 128]
            fb = pk[:, _FB:_FB + 16].bitcast(F32)
            bhn, bn1, bn0 = fb[:, 0:1], fb[:, 1:2], fb[:, 2:3]
            br1, bz1 = fb[:, 3:4], fb[:, 4:5]
            br0, bz0 = fb[:, 5:6], fb[:, 6:7]
            wyr = pk[0:1, _WYR:_WYR + 128]
            wyz = pk[0:1, _WYZ:_WYZ + 128]
            wyn = pk[0:1, _WYN:_WYN + 128]
            y0ab = pk[0:1, _Y0AB:_Y0AB + 512]

            wft = wpool.tile([128, 384], F16)
            w1t = wpool.tile([128, 256], F16)
            whhnt = wpool.tile([128, 128], F16)
            wynt = wpool.tile([128, 128], F16)
            whhrz0 = wpool.tile([128, 256], F16)
            for sb, dr in [
                (wft, wft_d), (w1t, w1t_d), (whhnt, whhnt_d),
                (wynt, wynt_d), (whhrz0, whhrz0_d),
            ]:
                nc.sync.dma_start(sb[:], dr[:])
            h0sb = wpool.tile([128, BS], F16)
            nc.sync.dma_start(h0sb[:], h0T_d[:])

            hprev = [h0sb[:, 0:C], h0sb[:, C:2 * C]]

            # shared psum banks
            ghn = ps_ghn.tile([128, 512], F32)
            pus = ps_u.tile([128, 512], F32)

            # ---- HAM warm-up: dense dummy matmuls into pus ----
            for _ in range(NWARM):
                nc.tensor.matmul(pus[:], ident, pk[:, 0:512],
                                 start=True, stop=True)

            # Zero-weight filler matmul: accumulates 0 into an open psum
            # generation. Keeps the PE streaming through stall windows so
            # the HAM clock gate stays at 8/8 (2.4 GHz).
            zlhs = pk[32:33, _WYR:_WYR + 128]  # all-zero row

            def pe_fill(dst, n=1):
                # rhs must avoid the fp32-bitcast _FB region: f16 views of
                # fp32 bytes can be NaN and 0*NaN = NaN.
                for _ in range(n):
                    nc.tensor.matmul(dst, zlhs, pk[32:33, _WYR:_WYR + 512],
                                     start=False, stop=False,
                                     skip_group_check=True)

            # ---- feats chunks ----
            fchunks = {}

            def load_chunk(ci):
                p0 = ci * ch
                pn_ = min(ch, npairs - p0)
                ft = fpool.tile([128, ch * 512], F16, tag="ft")
                nc.sync.dma_start(ft[:, :pn_ * 512], featsT[:, p0:p0 + pn_, :])
                fchunks[ci] = ft

            def fh_of(tt):
                p = tt // 2
                ci, po = divmod(p, ch)
                half = (tt % 2) * 64
                return fchunks[ci][half:half + 64, po * 512:(po + 1) * 512], half

            prs, pzs, pns = {}, {}, {}
            r16s = [None, None]
            z16s = [None, None]
            n16s = [None, None]
            zcs = [None, None]
            zhs = [None, None]

            def emit_feats_pair(t):
                # per-gate feats matmuls for steps (t, t+1), both chains in
                # one N=512 matmul; even t rows 0:64, odd rows 64:128.
                # Gate-major order alternates PE row groups so consecutive
                # matmuls overlap in the array.
                # The sim's zero-region group checker can't express "open
                # bank, per-chain sub-region accumulates, reads of finished
                # halves", but per-element has_written bits make it correct
                # on HW -> skip_group_check on every pr/pz/pn matmul.
                tts = [tt for tt in (t, t + 1) if tt < nt]
                gens = {}
                for tt in tts:
                    pr = ps_r.tile([128, 512], F32, tag="pr", name=f"pr{tt}")
                    pz = ps_z.tile([128, 512], F32, tag="pz", name=f"pz{tt}")
                    pn = ps_n.tile([128, 512], F32, tag="pn", name=f"pn{tt}")
                    gens[tt] = (pr, pz, pn)
                for gi, w0 in ((0, 0), (1, 128), (2, 256)):
                    for tt in tts:
                        fh, half = fh_of(tt)
                        nc.tensor.matmul(gens[tt][gi][:],
                                         wft[half:half + 64, w0:w0 + 128], fh,
                                         start=True, stop=False,
                                         tile_position=(half, 0),
                                         skip_group_check=True)
                if 0 in gens:
                    for gi, wy in ((0, wyr), (1, wyz), (2, wyn)):
                        nc.tensor.matmul(gens[0][gi][:], wy, y0ab, start=False,
                                         stop=False, skip_group_check=True)
                for tt in tts:
                    prs[tt], pzs[tt], pns[tt] = gens[tt]

            def stage_a(c2, t):
                # recurrent matmul burst + per-gate sigmoids
                hp = hprev[c2]
                cs = slice(c2 * C, (c2 + 1) * C)
                wrz = whhrz0 if t == 0 else w1t
                # filler runs while the PE waits for h'(t-1) from the DVE
                pe_fill(prs[t][:], NFILL_A)
                nc.tensor.matmul(prs[t][:, cs], wrz[:, 0:128], hp,
                                 start=False, stop=(c2 == 1),
                                 skip_group_check=True)
                nc.tensor.matmul(pzs[t][:, cs], wrz[:, 128:256], hp,
                                 start=False, stop=(c2 == 1),
                                 skip_group_check=True)
                nc.tensor.matmul(ghn[:, cs], whhnt[:], hp,
                                 start=True, stop=True)
                if t > 0:
                    nc.tensor.matmul(pns[t][:, cs], wynt[:], hp,
                                     start=False, stop=False,
                                     skip_group_check=True)
                r16 = gpool.tile([128, C], F16, tag=f"r16{c2}")
                nc.scalar.activation(r16[:], prs[t][:, cs], AF.Sigmoid,
                                     bias=br0 if t == 0 else br1)
                z16 = gpool.tile([128, C], F16, tag=f"z16{c2}")
                nc.scalar.activation(z16[:], pzs[t][:, cs], AF.Sigmoid,
                                     bias=bz0 if t == 0 else bz1)
                r16s[c2] = r16
                z16s[c2] = z16
                if c2 == 1:
                    prs.pop(t)
                    pzs.pop(t)

            def stage_b(c2, t):
                # t1, ident-accumulate, tanh, zc/zh
                cs = slice(c2 * C, (c2 + 1) * C)
                t1 = gpool.tile([128, C], F16, tag=f"t1{c2}")
                nc.vector.scalar_tensor_tensor(
                    t1[:], ghn[:, cs], bhn, r16s[c2][:], ALU.add, ALU.mult)
                pn = pns[t]
                # filler runs while the PE waits for t1 from the DVE
                pe_fill(pn[:], NFILL_B)
                nc.tensor.matmul(pn[:, cs], ident, t1[:],
                                 start=False, stop=(c2 == 1),
                                 skip_group_check=True)
                n16 = gpool.tile([128, C], F16, tag=f"n16{c2}")
                nc.scalar.activation(n16[:], pn[:, cs], AF.Tanh,
                                     bias=bn0 if t == 0 else bn1)
                n16s[c2] = n16
                zc = gpool.tile([128, C], F16, tag=f"zc{c2}")
                nc.gpsimd.tensor_scalar(zc[:], z16s[c2][:], -1.0, 1.0,
                                        ALU.mult, ALU.add)
                zh = gpool.tile([128, C], F16, tag=f"zh{c2}")
                nc.gpsimd.tensor_tensor(zh[:], z16s[c2][:], hprev[c2], ALU.mult)
                zcs[c2] = zc
                zhs[c2] = zh
                if c2 == 1:
                    pns.pop(t)

            def stage_c(c2, t):
                # combine h' and the y matmul; evac every 4 steps
                zn = gpool.tile([128, C], F16, tag=f"zn{c2}")
                nc.vector.tensor_tensor(zn[:], zcs[c2][:], n16s[c2][:], ALU.mult)
                hT = hpool.tile([128, C], F16, tag=f"h{c2}")
                nc.vector.tensor_tensor(hT[:], zn[:], zhs[c2][:], ALU.add)
                hprev[c2] = hT
                c4 = t % 4
                nc.tensor.matmul(
                    pus[32 * c4:32 * (c4 + 1), c2 * C:(c2 + 1) * C], woc, hT[:],
                    start=True, stop=True, tile_position=(0, 32 * c4),
                )
                if c4 == 3 and c2 == 1:
                    g = t // 4
                    yf = ypool.tile([128, 512], F16, tag="yf")
                    nc.vector.tensor_copy(yf[:], pus[:])
                    nc.sync.dma_start(yT[4 * g:4 * (g + 1), :], yf[0:128:32, :])

            # ---- prologue ----
            load_chunk(0)
            if nchunk > 1:
                load_chunk(1)
            emit_feats_pair(0)

            # ---- steady loop ----
            for t in range(nt):
                if t % 2 == 0 and t > 0:
                    p = t // 2
                    ci = p // ch
                    if p % ch == 0 and ci + 1 < nchunk:
                        load_chunk(ci + 1)
                    emit_feats_pair(t)
                if t > 0:
                    stage_b(1, t - 1)
                    stage_c(0, t - 1)
                stage_a(0, t)
                if t > 0:
                    stage_c(1, t - 1)
                stage_a(1, t)
                stage_b(0, t)

            # ---- tail ----
            stage_b(1, nt - 1)
            stage_c(0, nt - 1)
            stage_c(1, nt - 1)

    nc.compile()
    return nc


# -------- host-side weight prep + sharded execution --------

def _prep_aux(W_ih, W_hh, b_ih, b_hh, Wo, bo):
    W_ih = np.asarray(W_ih, np.float32)
    W_hh = np.asarray(W_hh, np.float32)
    b_ih = np.asarray(b_ih, np.float32)
    b_hh = np.asarray(b_hh, np.float32)
    wo = np.asarray(Wo, np.float32)[0]       # [H]
    bo_s = float(np.asarray(bo, np.float32)[0])
    wfd = W_ih[:, :D]                         # [3H, D]
    w_y = W_ih[:, D]                          # [3H]

    wft = np.zeros((128, 384), np.float16)
    wft[0:64] = wfd.T.astype(np.float16)
    wft[64:128] = wfd.T.astype(np.float16)

    W1 = W_hh[0:2 * H] + np.outer(w_y[0:2 * H], wo)       # [2H, H]
    aux = dict(
        wft=wft,
        w1t=np.ascontiguousarray(W1.T.astype(np.float16)),
        whhnt=np.ascontiguousarray(W_hh[2 * H:].T.astype(np.float16)),
        wynt=np.ascontiguousarray(np.outer(wo, w_y[2 * H:]).astype(np.float16)),
        whhrz0=np.ascontiguousarray(W_hh[0:2 * H].T.astype(np.float16)),
    )

    pk = np.zeros((128, NPACK), np.float16)
    pk[:, _WOC0:_WOC0 + 32] = np.repeat(wo[:, None], 32, axis=1).astype(np.float16)
    pk[:, _ID0:_ID0 + 128] = np.eye(128, dtype=np.float16)
    brz_base = (b_ih + b_hh)[0:2 * H]
    fbv = np.stack(
        [b_hh[2 * H:],                            # bhn
         b_ih[2 * H:] + w_y[2 * H:] * bo_s,       # bn1
         b_ih[2 * H:],                            # bn0
         brz_base[0:H] + w_y[0:H] * bo_s,         # br1
         brz_base[H:2 * H] + w_y[H:2 * H] * bo_s,  # bz1
         brz_base[0:H],                           # br0
         brz_base[H:2 * H]],                      # bz0
        axis=1,
    ).astype(np.float32)
    fb16 = np.zeros((128, 16), np.float16)
    fb16[:, 0:14] = fbv.view(np.float16)
    pk[:, _FB:_FB + 16] = fb16
    pk[0, _WYR:_WYR + 128] = w_y[0:H].astype(np.float16)
    pk[0, _WYZ:_WYZ + 128] = w_y[H:2 * H].astype(np.float16)
    pk[0, _WYN:_WYN + 128] = w_y[2 * H:].astype(np.float16)
    aux["pack"] = pk
    aux["bo_s"] = bo_s
    return aux


def _core_featsT(ff_core):
    """[BS, nt, D] fp16 -> [128, nt//2, 512]:
    rows = (t%2)*64 + d, cols = c2*256 + batch-within-chain."""
    nt = ff_core.shape[1]
    a = ff_core.reshape(2, C, nt // 2, 2, D)   # [c2, cb, p, par, d]
    a = a.transpose(3, 4, 2, 0, 1)             # [par, d, p, c2, cb]
    return np.ascontiguousarray(a).reshape(128, nt // 2, 512)


def _fill_y0(pkc, y0c):
    """Write per-core y0 (fp16 [BS]) into the pack's y0 row."""
    pkc[0, _Y0AB:_Y0AB + 512] = y0c


_NC_CACHE = {}


def kernel(future_feats, h0, y0, W_ih, W_hh, b_ih, b_hh, Wo, bo):
    ff = np.asarray(future_feats).astype(np.float16)      # [B, T, D]
    h0f = np.asarray(h0).astype(np.float16)[0]            # [B, H]
    y0f = np.asarray(y0).astype(np.float16)               # [B]

    aux = _prep_aux(W_ih, W_hh, b_ih, b_hh, Wo, bo)
    bo_s = aux.pop("bo_s")

    if "nc" not in _NC_CACHE:
        _NC_CACHE["nc"] = build(T)
    nc = _NC_CACHE["nc"]

    in_maps = []
    for c in range(NCORES):
        sl = slice(c * BS, (c + 1) * BS)
        m = dict(aux)
        pkc = aux["pack"].copy()
        _fill_y0(pkc, y0f[sl])
        m["pack"] = pkc
        m["featsT"] = _core_featsT(ff[sl])
        m["h0T"] = np.ascontiguousarray(h0f[sl].T)
        in_maps.append(m)

    res = run_bass_kernel_spmd(nc, in_maps, core_ids=list(range(NCORES)))
    outs = [r["yT"] for r in res.results]
    out = np.concatenate([o.T.astype(np.float32) for o in outs], axis=0)
    return out + bo_s
